# revision 49
# baseline (speedup 1.0000x reference)
"""Trainium2 Bass kernel for nn_Graphs (soft decision-graph probability propagation).

Reference math (G=4 graphs, B=128 batch, N=255 internal nodes, L=256 leaves,
F=512 features, J=8 jumps):
  b  = sigmoid(x @ W_g^T + bias_g)                  (per graph: B x N)
  M0 = softmax(M_left, axis=dest), M1 = softmax(M_right, axis=dest)
  q  = [b*(M1-M0)+M0 | leaf-identity]               (per (g,batch): 511x511)
  prob <- q @ prob, J times, starting from e0; return leaf probs.

Restructure (v1):
  - q never materialized. With u = prob[internal], one jump is
      u' = E0 @ (c0*u) + E1 @ (c1*u),  c0 = r0*(1-b), c1 = r1*b
    where E0/E1 = exp(M^T) (softmax denominators r folded into c).
  - Leaf rows only accumulate:  leaf = E0L @ (sum_j c0*u_j) + E1L @ (sum_j c1*u_j)
    -> the leaf matmuls are hoisted OUT of the jump loop entirely; the
    running sums (supv) are kept by the otherwise-idle gpsimd engine.
  - Jump 0's state is the one-hot e0, so its 8 matmuls collapse to 4
    rank-1 matmuls reading row 0 of c01 directly.
  - bias is folded into the b-matmul as a rank-1 (bias x ones) accumulate,
    removing the bias DMA + activation bias dependency.

Scheduling (v1):
  - DMA issue (DIRECT2D ~650ns/instr, serialized per sequencer) is split
    across BOTH hwdge rings: Act ring carries the two src-half-0 M tiles
    (needed first by the exp stream), SP ring carries wxb + the two
    src-half-1 M tiles. gpsimd issues no DMAs (avoids the 5us SWDGE drain).
  - A dummy activation triggers the ACT_TABLE_LOAD at t~0 so the exp
    stream starts the moment the first M tile lands.
  - Row sums run on DVE (tensor_reduce) instead of the Act accumulator,
    cutting the serial Act-engine time by ~1.5us.
  - PE warm-up matmuls (bf16, no deps) keep the PE clock ramped through
    the prefix; tuned so the jump chain runs at the 2.4GHz p-state.
  - Generous tile-pool buffering (no tag reuse for one-shot tiles) keeps
    instructions at <=1 semaphore wait, minimizing bacc event semaphores
    (the end-of-program event-sem drain is ~150ns per event sem).

Sharding: 8 cores = (graph g = core//2) x (batch half h = core%2, 64 rows).
Output per core: (64,256) batch-major leaf probs; host assembles (B,L,G)
and applies the reference interval clamp.
"""

import numpy as np

G, B, N, L, F, J = 4, 128, 255, 256, 512, 8
BH = B // 2  # 64 batch rows per core
NCORES = 8
NEG = np.float32(-1e4)

_CACHE = {}

# PE warm-up budget (tuned from traces): bf16 512-row matmuls ~216ns at peak.
WARM1 = 4   # before the b-phase matmuls
WARM2 = 4   # between b-phase and jump 0
WARM3 = 6   # between jump 1's src-half-0 and src-half-1 matmuls


def _build_program():
    import concourse.mybir as mybir
    from concourse import bacc
    from concourse.tile import TileContext

    f32 = mybir.dt.float32
    bf16 = mybir.dt.bfloat16
    f32r = mybir.dt.float32r  # single-pass fp32 matmul mode
    AF = mybir.ActivationFunctionType
    AX = mybir.AxisListType
    mult = mybir.AluOpType.mult
    addop = mybir.AluOpType.add

    nc = bacc.Bacc(None)
    p_mlt = nc.declare_dram_parameter("mlt", [256, 512], f32, isOutput=False)
    p_mrt = nc.declare_dram_parameter("mrt", [256, 512], f32, isOutput=False)
    p_wxb = nc.declare_dram_parameter("wxb", [128, 1280], f32r, isOutput=False)
    p_bp2 = nc.declare_dram_parameter("bp2", [128, 2], f32, isOutput=False)
    p_idn = nc.declare_dram_parameter("idn", [128, 128], f32r, isOutput=False)
    p_out = nc.declare_dram_parameter("out", [BH, 256], f32, isOutput=True)

    # dtype discipline (measured on hw): f32r data is exact ONLY through the
    # PE; any DVE/gpsimd/Act READ of f32r degrades to ~bf16. So f32r tiles
    # (wxb, el/er, upv, sup) are consumed exclusively by matmuls; every
    # value another engine needs (row sums via accum_out, pq in PSUM) stays
    # plain f32.
    def rmm(out, lhsT, rhs, **kw):
        nc.tensor.matmul(out, lhsT, rhs, **kw)

    with TileContext(nc) as tc:
        with (
            tc.tile_pool(name="consts", bufs=1) as consts,
            tc.tile_pool(name="work", bufs=1) as work,
            tc.tile_pool(name="state", bufs=7) as state,
            tc.tile_pool(name="psum", bufs=1, space="PSUM") as psum,
            tc.tile_pool(name="psum1", bufs=1, space="PSUM") as psum1,
        ):
            # ---- early consts (gpsimd memsets; keeps DVE/Act/PE free) ----
            dumw = consts.tile([128, 1], f32, tag="dumw", name="dumw")
            nc.gpsimd.memset(dumw[:], 0.0)
            wsc = consts.tile([128, 128], bf16, tag="wsc", name="wsc")
            rsc = consts.tile([128, 512], bf16, tag="rsc", name="rsc")
            nc.gpsimd.memset(wsc[:], 0.0)
            nc.gpsimd.memset(rsc[:], 0.0)
            u32 = mybir.dt.uint32
            # jump-0 state (only src row 0 non-zero; src half 1 is all zero
            # and its matmuls are skipped). f32r: read only by the PE.
            upv0 = state.tile([128, 2, BH], f32r, tag="u0seed", name="upv0",
                              bufs=1)
            nc.gpsimd.memset(upv0[:].bitcast(u32), 0)

            # trigger the ACT_TABLE_LOAD for Exp immediately (runs on the Act
            # ENGINE while the Act SEQUENCER generates the DMA descriptors
            # below)
            dumo = work.tile([128, 1], f32, tag="dumo", name="dumo")
            nc.scalar.activation(dumo[:], dumw[:], AF.Exp)

            # ---- input DMAs, split across both hwdge rings ----
            # eraw index: 0 = el src-half0, 1 = el src-half1, 2 = er h0, 3 = er h1
            eraw = [consts.tile([128, 512], f32, tag=f"eraw{i}", name=f"eraw{i}")
                    for i in range(4)]
            nc.scalar.dma_start(eraw[0][:], p_mlt[0:128, :])
            nc.scalar.dma_start(eraw[2][:], p_mrt[0:128, :])
            bp2 = consts.tile([128, 2], f32, tag="bp2", name="bp2")
            nc.scalar.dma_start(bp2[:], p_bp2[:, :])
            idn = consts.tile([128, 128], f32r, tag="idn", name="idn")
            nc.scalar.dma_start(idn[:], p_idn[:, :])
            wxb = consts.tile([128, 1280], f32r, tag="wxb", name="wxb")
            nc.sync.dma_start(wxb[:], p_wxb[:, :])
            nc.sync.dma_start(eraw[1][:], p_mlt[128:256, :])
            nc.sync.dma_start(eraw[3][:], p_mrt[128:256, :])

            # ---- PE warm-up ----
            pwarm = psum1.tile([128, 512], f32, tag="pwarm", name="pwarm")

            def warm(n):
                for _ in range(n):
                    nc.tensor.matmul(pwarm[:], wsc[:], rsc[:], start=True, stop=True)

            # ---- exp(M^T), lazily normalized ----
            el = [consts.tile([128, 512], f32r, tag=f"el{t}", name=f"el{t}")
                  for t in range(2)]
            er = [consts.tile([128, 512], f32r, tag=f"er{t}", name=f"er{t}")
                  for t in range(2)]

            # exp with inline row-sum accumulation (Act's accumulator is
            # fp32-exact; the f32r-tagged exp output can't be summed by DVE)
            psums = {}

            def exps(dst, src, i):
                ps = [work.tile([128, 1], f32, tag=f"ps{i}_{hh}",
                                name=f"ps{i}_{hh}") for hh in range(2)]
                for hh in range(2):
                    sl = slice(hh * 256, (hh + 1) * 256)
                    nc.scalar.activation(dst[:, sl], src[:, sl], AF.Exp,
                                         accum_out=ps[hh][:])
                psums[i] = ps

            # Act order: src-half-0 tiles first (gate jump 0), then eb
            # (b-phase), then src-half-1 tiles (gate jump 1's back half).
            exps(el[0], eraw[0], 0)
            exps(er[0], eraw[2], 2)

            # ---- b-phase on PE (also part of clock ramp) ----
            warm(WARM1)
            pbb = psum.tile([128, 2, BH], f32, tag="pbb", name="pbb")
            for mh in range(2):
                for k in range(4):
                    rmm(pbb[:, mh, :],
                        wxb[:, k * 320 + mh * 128:k * 320 + (mh + 1) * 128],
                        wxb[:, k * 320 + 256:k * 320 + 320],
                        start=(k == 0), stop=(k == 3))

            # eb = exp(-(logit)) = exp(pb*-1 + (-bias)); bp2 carries -bias
            eb = [work.tile([128, BH], f32, tag=f"eb{mh}", name=f"eb{mh}")
                  for mh in range(2)]
            for mh in range(2):
                nc.scalar.activation(eb[mh][:], pbb[:, mh, :], AF.Exp,
                                     bias=bp2[:, mh:mh + 1], scale=-1.0)

            exps(el[1], eraw[1], 1)
            exps(er[1], eraw[3], 3)

            # ---- reciprocals of row sums on DVE; c01 packing ----
            # c01[mh][:,0] = r0*(1-b), c01[mh][:,1] = r1*b
            def recip_of(i):
                ps = psums[i]
                red = work.tile([128, 1], f32, tag=f"red{i}", name=f"red{i}")
                nc.vector.tensor_add(red[:], ps[0][:], ps[1][:])
                r = consts.tile([128, 1], f32, tag=f"rec{i}", name=f"rec{i}")
                nc.vector.reciprocal(r[:], red[:])
                return r

            c01 = [consts.tile([128, 2, BH], f32, tag=f"c01{t}", name=f"c01{t}")
                   for t in range(2)]

            def c01_chain(mh, r0, r1):
                den = work.tile([128, BH], f32, tag=f"den{mh}", name=f"den{mh}")
                nc.vector.tensor_scalar_add(den[:], eb[mh][:], 1.0)
                sig = work.tile([128, BH], f32, tag=f"sig{mh}", name=f"sig{mh}")
                nc.vector.reciprocal(sig[:], den[:])
                nc.vector.tensor_scalar_mul(c01[mh][:, 1], sig[:], r1[:])
                nc.vector.tensor_mul(sig[:], sig[:], eb[mh][:])
                nc.vector.tensor_scalar_mul(c01[mh][:, 0], sig[:], r0[:])

            rec0 = recip_of(0)
            rec2 = recip_of(2)
            c01_chain(0, rec0, rec2)

            # jump-0 state: row 0 of c01[0] (DVE rounds it to f32r for the PE)
            nc.vector.tensor_copy(upv0[0:1, :, :], c01[0][0:1, :, :])

            # sum_j upv_j is accumulated EXACTLY in PSUM by identity matmuls
            # (0/1 weights are exact in the PE's f32r path; u_j decays with j
            # so, unlike a prefix-sum recursion, fp noise does not amplify)
            psup = [psum.tile([128, 2, BH], f32, tag=f"psup{t}",
                              name=f"psup{t}") for t in range(2)]
            sup_started = [False, False]

            def sup_acc(upv_t, t, stop=False):
                rmm(psup[t][:], idn[:], upv_t[:],
                    start=not sup_started[t], stop=stop)
                sup_started[t] = True

            # ---- jump 0 on PE: only src-half-0 of u_0 is non-zero ----
            warm(WARM2)
            pq = psum.tile([128, 2, BH], f32, tag="pq", name="pq", bufs=3)
            for mt in range(2):
                ms = slice(mt * 128, (mt + 1) * 128)
                rmm(pq[:, mt, :], el[0][:, ms], upv0[:, 0, :],
                    start=True, stop=False)
                rmm(pq[:, mt, :], er[0][:, ms], upv0[:, 1, :],
                    start=False, stop=True)
            sup_acc(upv0, 0)

            # DVE: upv_1 src-half-0 can go as soon as pq_0 lands
            upv = [state.tile([128, 2, BH], f32r, tag=f"upv{t}", name=f"upv{t}")
                   for t in range(2)]
            nc.vector.tensor_tensor(
                out=upv[0][:], in0=c01[0][:],
                in1=pq[:, 0, :][:, None, :].broadcast_to([128, 2, BH]), op=mult)

            rec1 = recip_of(1)
            rec3 = recip_of(3)
            c01_chain(1, rec1, rec3)
            nc.vector.tensor_tensor(
                out=upv[1][:], in0=c01[1][:],
                in1=pq[:, 1, :][:, None, :].broadcast_to([128, 2, BH]), op=mult)

            # ---- jumps 1..6: pq_1 .. pq_6 (u_2 .. u_7) ----
            # per jump: 8 matmuls (2 dst halves x 2 src halves x 2 matrices)
            # + 2 identity matmuls accumulating this jump's upv into psup;
            # front 4 need only upv[src-half-0]
            for j in range(1, J - 1):
                pq_new = psum.tile([128, 2, BH], f32, tag="pq", name="pq",
                                   bufs=3)
                if j == 1:
                    warm(WARM3)
                for mt in range(2):
                    ms = slice(mt * 128, (mt + 1) * 128)
                    rmm(pq_new[:, mt, :], el[0][:, ms], upv[0][:, 0],
                        start=True, stop=False)
                    rmm(pq_new[:, mt, :], er[0][:, ms], upv[0][:, 1],
                        start=False, stop=False)
                    rmm(pq_new[:, mt, :], el[1][:, ms], upv[1][:, 0],
                        start=False, stop=False)
                    rmm(pq_new[:, mt, :], er[1][:, ms], upv[1][:, 1],
                        start=False, stop=True)
                # this jump's state joins the running sum (identity weights
                # stay loaded across the consecutive pair)
                sup_acc(upv[0], 0)
                sup_acc(upv[1], 1)

                upv_new = [state.tile([128, 2, BH], f32r, tag=f"upv{t}",
                                      name=f"upv{t}") for t in range(2)]
                nc.vector.tensor_tensor(
                    out=upv_new[0][:], in0=c01[0][:],
                    in1=pq_new[:, 0, :][:, None, :].broadcast_to([128, 2, BH]),
                    op=mult)
                nc.vector.tensor_tensor(
                    out=upv_new[1][:], in0=c01[1][:],
                    in1=pq_new[:, 1, :][:, None, :].broadcast_to([128, 2, BH]),
                    op=mult)
                upv = upv_new
                pq = pq_new

            # final state u_7 closes both accumulators
            sup_acc(upv[0], 0, stop=True)
            sup_acc(upv[1], 1, stop=True)

            # ---- leaf block: leaf = C @ sum_j u_j via sup = sum_j upv_j ----
            supc = [state.tile([128, 2, BH], f32r, tag=f"supc{t}",
                               name=f"supc{t}", bufs=1) for t in range(2)]
            for t in range(2):
                nc.vector.tensor_copy(supc[t][:], psup[t][:])
            pleaf = psum1.tile([BH, 256], f32, tag="pl", name="pl")
            rmm(pleaf[:], supc[0][:, 0], el[0][:, 256:512], start=True, stop=False)
            rmm(pleaf[:], supc[1][:, 0], el[1][:, 256:512], start=False, stop=False)
            rmm(pleaf[:], supc[0][:, 1], er[0][:, 256:512], start=False, stop=False)
            rmm(pleaf[:], supc[1][:, 1], er[1][:, 256:512], start=False, stop=True)

            # ---- output ----
            o = work.tile([BH, 256], f32, tag="o", name="o")
            nc.scalar.copy(o[:], pleaf[:])
            nc.sync.dma_start(p_out[:, :], o[:])

    nc.finalize()
    return nc


def _get_program():
    if "nc" not in _CACHE:
        _CACHE["nc"] = _build_program()
    return _CACHE["nc"]


def _prep_inputs(x, W, bias, M_left, M_right):
    """Host-side shard + layout prep. Core c -> graph c//2, batch half c%2."""
    in_maps = []
    mlt_g, mrt_g, wt_g = [], [], []
    for g in range(G):
        mlt = np.zeros((256, 512), np.float32)
        mrt = np.zeros((256, 512), np.float32)
        tl = np.ascontiguousarray(M_left[g].T)   # (255, 511)
        tr = np.ascontiguousarray(M_right[g].T)
        for dst, src in ((mlt, tl), (mrt, tr)):
            dst[0:255, 0:255] = src[:, 0:255]
            dst[0:255, 256:512] = src[:, 255:511]
            dst[0:255, 255] = NEG
        mlt_g.append(mlt)
        mrt_g.append(mrt)
        wt = np.zeros((512, 256), np.float32)
        wt[:, 0:255] = W[g].T
        wt_g.append(wt)
    xt_h = [np.ascontiguousarray(x[h * BH:(h + 1) * BH].T) for h in range(2)]
    for c in range(NCORES):
        g, h = c // 2, c % 2
        wxb = np.zeros((128, 1280), np.float32)
        wt, xt = wt_g[g], xt_h[h]
        for k in range(4):
            wxb[:, k * 320:k * 320 + 256] = wt[k * 128:(k + 1) * 128, :]
            wxb[:, k * 320 + 256:k * 320 + 320] = xt[k * 128:(k + 1) * 128, :]
        bp2 = np.zeros((128, 2), np.float32)
        bp2[:, 0] = -bias[g][0:128]
        bp2[0:127, 1] = -bias[g][128:255]
        in_maps.append({"mlt": mlt_g[g], "mrt": mrt_g[g], "wxb": wxb,
                        "bp2": bp2, "idn": np.eye(128, dtype=np.float32)})
    return in_maps


def _assemble(results):
    eps = np.float32(1e-5)
    ret = np.empty((B, L, G), np.float32)
    for c in range(NCORES):
        g, h = c // 2, c % 2
        ret[h * BH:(h + 1) * BH, :, g] = results[c]["out"]
    ret = np.where(ret > 0.0, ret, eps)
    ret = np.where(ret < 1.0, ret, np.float32(1.0) - eps)
    return ret.astype(np.float32)


def run_on_device(in_maps, trace=False, **kw):
    from concourse.bass_utils import run_bass_kernel_spmd
    nc = _get_program()
    return run_bass_kernel_spmd(nc, in_maps, list(range(NCORES)), trace=trace, **kw)


def kernel(x, W, bias, M_left, M_right):
    in_maps = _prep_inputs(
        np.asarray(x, np.float32), np.asarray(W, np.float32),
        np.asarray(bias, np.float32), np.asarray(M_left, np.float32),
        np.asarray(M_right, np.float32),
    )
    res = run_on_device(in_maps)
    return _assemble(res.results)


# revision 55
# speedup vs baseline: 1.0412x; 1.0412x over previous
"""Trainium2 Bass kernel for nn_Graphs (soft decision-graph probability propagation).

Reference math (G=4 graphs, B=128 batch, N=255 internal nodes, L=256 leaves,
F=512 features, J=8 jumps):
  b  = sigmoid(x @ W_g^T + bias_g)                  (per graph: B x N)
  M0 = softmax(M_left, axis=dest), M1 = softmax(M_right, axis=dest)
  q  = [b*(M1-M0)+M0 | leaf-identity]               (per (g,batch): 511x511)
  prob <- q @ prob, J times, starting from e0; return leaf probs.

Restructure (v1):
  - q never materialized. With u = prob[internal], one jump is
      u' = E0 @ (c0*u) + E1 @ (c1*u),  c0 = r0*(1-b), c1 = r1*b
    where E0/E1 = exp(M^T) (softmax denominators r folded into c).
  - Leaf rows only accumulate:  leaf = E0L @ (sum_j c0*u_j) + E1L @ (sum_j c1*u_j)
    -> the leaf matmuls are hoisted OUT of the jump loop entirely; the
    running sums (supv) are kept by the otherwise-idle gpsimd engine.
  - Jump 0's state is the one-hot e0, so its 8 matmuls collapse to 4
    rank-1 matmuls reading row 0 of c01 directly.
  - bias is folded into the b-matmul as a rank-1 (bias x ones) accumulate,
    removing the bias DMA + activation bias dependency.

Scheduling (v1):
  - DMA issue (DIRECT2D ~650ns/instr, serialized per sequencer) is split
    across BOTH hwdge rings: Act ring carries the two src-half-0 M tiles
    (needed first by the exp stream), SP ring carries wxb + the two
    src-half-1 M tiles. gpsimd issues no DMAs (avoids the 5us SWDGE drain).
  - A dummy activation triggers the ACT_TABLE_LOAD at t~0 so the exp
    stream starts the moment the first M tile lands.
  - Row sums run on DVE (tensor_reduce) instead of the Act accumulator,
    cutting the serial Act-engine time by ~1.5us.
  - PE warm-up matmuls (bf16, no deps) keep the PE clock ramped through
    the prefix; tuned so the jump chain runs at the 2.4GHz p-state.
  - Generous tile-pool buffering (no tag reuse for one-shot tiles) keeps
    instructions at <=1 semaphore wait, minimizing bacc event semaphores
    (the end-of-program event-sem drain is ~150ns per event sem).

Sharding: 8 cores = (graph g = core//2) x (batch half h = core%2, 64 rows).
Output per core: (64,256) batch-major leaf probs; host assembles (B,L,G)
and applies the reference interval clamp.
"""

import numpy as np

G, B, N, L, F, J = 4, 128, 255, 256, 512, 8
BH = B // 2  # 64 batch rows per core
NCORES = 8
NEG = np.float32(-1e4)

_CACHE = {}

# PE warm-up budget (tuned from traces): bf16 512-row matmuls ~216ns at peak.
# The tile scheduler interleaves these dependency-free matmuls into PE idle
# gaps, keeping the clock ramped until the jump chain starts.
WARM1 = 14  # before the b-phase matmuls
WARM2 = 12  # between b-phase and jump 0


def _build_program():
    import concourse.mybir as mybir
    from concourse import bacc
    from concourse.tile import TileContext

    f32 = mybir.dt.float32
    bf16 = mybir.dt.bfloat16
    f32r = mybir.dt.float32r  # single-pass fp32 matmul mode
    AF = mybir.ActivationFunctionType
    AX = mybir.AxisListType
    mult = mybir.AluOpType.mult
    addop = mybir.AluOpType.add

    nc = bacc.Bacc(None)
    p_mlt = nc.declare_dram_parameter("mlt", [256, 512], f32, isOutput=False)
    p_mrt = nc.declare_dram_parameter("mrt", [256, 512], f32, isOutput=False)
    p_wxb = nc.declare_dram_parameter("wxb", [128, 1280], f32r, isOutput=False)
    p_bp2 = nc.declare_dram_parameter("bp2", [128, 2], f32, isOutput=False)
    p_idn = nc.declare_dram_parameter("idn", [128, 128], f32r, isOutput=False)
    p_out = nc.declare_dram_parameter("out", [BH, 256], f32, isOutput=True)

    # dtype discipline (measured on hw): f32r data is exact ONLY through the
    # PE; any DVE/gpsimd/Act READ of f32r degrades to ~bf16. So f32r tiles
    # (wxb, el/er, upv, sup) are consumed exclusively by matmuls; every
    # value another engine needs (row sums via accum_out, pq in PSUM) stays
    # plain f32.
    def rmm(out, lhsT, rhs, **kw):
        nc.tensor.matmul(out, lhsT, rhs, **kw)

    with TileContext(nc) as tc:
        with (
            tc.tile_pool(name="consts", bufs=1) as consts,
            tc.tile_pool(name="work", bufs=1) as work,
            tc.tile_pool(name="state", bufs=7) as state,
            tc.tile_pool(name="psum", bufs=1, space="PSUM") as psum,
            tc.tile_pool(name="psum1", bufs=1, space="PSUM") as psum1,
        ):
            # ---- early consts (gpsimd memsets; keeps DVE/Act/PE free) ----
            dumw = consts.tile([128, 1], f32, tag="dumw", name="dumw")
            nc.gpsimd.memset(dumw[:], 0.0)
            wsc = consts.tile([128, 128], bf16, tag="wsc", name="wsc")
            rsc = consts.tile([128, 512], bf16, tag="rsc", name="rsc")
            nc.gpsimd.memset(wsc[:], 0.0)
            nc.gpsimd.memset(rsc[:], 0.0)
            u32 = mybir.dt.uint32
            # jump-0 state (only src row 0 non-zero; src half 1 is all zero
            # and its matmuls are skipped). f32r: read only by the PE.
            upv0 = state.tile([128, 2, BH], f32r, tag="u0seed", name="upv0",
                              bufs=1)
            nc.gpsimd.memset(upv0[:].bitcast(u32), 0)

            # trigger the ACT_TABLE_LOAD for Exp immediately (runs on the Act
            # ENGINE while the Act SEQUENCER generates the DMA descriptors
            # below)
            dumo = work.tile([128, 1], f32, tag="dumo", name="dumo")
            nc.scalar.activation(dumo[:], dumw[:], AF.Exp)

            # ---- input DMAs ----
            # Big transfers all on the SP hwdge ring (splitting across both
            # rings starves each; measured). Order = consumption order. Tiny
            # transfers ride the Act ring, whose issue overlaps the table
            # load. Each dma_start costs ~650ns of descriptor-gen on its
            # sequencer regardless of size, so M tiles go as full (128,512)
            # transfers.
            # eraw index: 0 = el src-half0, 1 = el src-half1, 2 = er h0, 3 = er h1
            eraw = [consts.tile([128, 512], f32, tag=f"eraw{i}", name=f"eraw{i}")
                    for i in range(4)]
            wxb = consts.tile([128, 1280], f32r, tag="wxb", name="wxb")
            nc.sync.dma_start(eraw[0][:], p_mlt[0:128, :])
            nc.sync.dma_start(eraw[2][:], p_mrt[0:128, :])
            nc.sync.dma_start(wxb[:], p_wxb[:, :])
            nc.sync.dma_start(eraw[1][:], p_mlt[128:256, :])
            nc.sync.dma_start(eraw[3][:], p_mrt[128:256, :])
            bp2 = consts.tile([128, 2], f32, tag="bp2", name="bp2")
            nc.scalar.dma_start(bp2[:], p_bp2[:, :])
            idn = consts.tile([128, 128], f32r, tag="idn", name="idn")
            nc.scalar.dma_start(idn[:], p_idn[:, :])

            # ---- PE warm-up ----
            pwarm = psum1.tile([128, 512], f32, tag="pwarm", name="pwarm")

            def warm(n):
                for _ in range(n):
                    nc.tensor.matmul(pwarm[:], wsc[:], rsc[:], start=True, stop=True)

            # ---- exp(M^T), lazily normalized ----
            el = [consts.tile([128, 512], f32r, tag=f"el{t}", name=f"el{t}")
                  for t in range(2)]
            er = [consts.tile([128, 512], f32r, tag=f"er{t}", name=f"er{t}")
                  for t in range(2)]

            # exp with inline row-sum accumulation (Act's accumulator is
            # fp32-exact; the f32r-tagged exp output can't be summed by DVE)
            psums = {}

            def exps(dst, src, i):
                ps = [work.tile([128, 1], f32, tag=f"ps{i}_{hh}",
                                name=f"ps{i}_{hh}") for hh in range(2)]
                for hh in range(2):
                    sl = slice(hh * 256, (hh + 1) * 256)
                    nc.scalar.activation(dst[:, sl], src[:, sl], AF.Exp,
                                         accum_out=ps[hh][:])
                psums[i] = ps

            # Act order: the four M tiles in DMA-arrival order, then eb (the
            # b-phase psum is long ready by then).
            exps(el[0], eraw[0], 0)
            exps(er[0], eraw[2], 2)
            exps(el[1], eraw[1], 1)
            exps(er[1], eraw[3], 3)

            # ---- b-phase on PE (also part of clock ramp) ----
            warm(WARM1)
            pbb = psum.tile([128, 2, BH], f32, tag="pbb", name="pbb")
            for mh in range(2):
                for k in range(4):
                    rmm(pbb[:, mh, :],
                        wxb[:, k * 320 + mh * 128:k * 320 + (mh + 1) * 128],
                        wxb[:, k * 320 + 256:k * 320 + 320],
                        start=(k == 0), stop=(k == 3))

            # eb = exp(-(logit)) = exp(pb*-1 + (-bias)); bp2 carries -bias
            eb = [work.tile([128, BH], f32, tag=f"eb{mh}", name=f"eb{mh}")
                  for mh in range(2)]
            for mh in range(2):
                nc.scalar.activation(eb[mh][:], pbb[:, mh, :], AF.Exp,
                                     bias=bp2[:, mh:mh + 1], scale=-1.0)

            # ---- reciprocals of row sums on DVE; c01 packing ----
            # c01[mh][:,0] = r0*(1-b), c01[mh][:,1] = r1*b
            def recip_of(i):
                ps = psums[i]
                red = work.tile([128, 1], f32, tag=f"red{i}", name=f"red{i}")
                nc.vector.tensor_add(red[:], ps[0][:], ps[1][:])
                r = consts.tile([128, 1], f32, tag=f"rec{i}", name=f"rec{i}")
                nc.vector.reciprocal(r[:], red[:])
                return r

            c01 = [consts.tile([128, 2, BH], f32, tag=f"c01{t}", name=f"c01{t}")
                   for t in range(2)]

            def c01_chain(mh, r0, r1):
                den = work.tile([128, BH], f32, tag=f"den{mh}", name=f"den{mh}")
                nc.vector.tensor_scalar_add(den[:], eb[mh][:], 1.0)
                sig = work.tile([128, BH], f32, tag=f"sig{mh}", name=f"sig{mh}")
                nc.vector.reciprocal(sig[:], den[:])
                nc.vector.tensor_scalar_mul(c01[mh][:, 1], sig[:], r1[:])
                nc.vector.tensor_mul(sig[:], sig[:], eb[mh][:])
                nc.vector.tensor_scalar_mul(c01[mh][:, 0], sig[:], r0[:])

            rec0 = recip_of(0)
            rec2 = recip_of(2)
            rec1 = recip_of(1)
            rec3 = recip_of(3)
            c01_chain(0, rec0, rec2)

            # jump-0 state: row 0 of c01[0] (DVE rounds it to f32r for the PE)
            nc.vector.tensor_copy(upv0[0:1, :, :], c01[0][0:1, :, :])
            c01_chain(1, rec1, rec3)

            # sum_j upv_j is accumulated EXACTLY in PSUM by identity matmuls
            # (0/1 weights are exact in the PE's f32r path; u_j decays with j
            # so, unlike a prefix-sum recursion, fp noise does not amplify)
            psup = [psum.tile([128, 2, BH], f32, tag=f"psup{t}",
                              name=f"psup{t}") for t in range(2)]
            sup_started = [False, False]

            def sup_acc(upv_t, t, stop=False):
                rmm(psup[t][:], idn[:], upv_t[:],
                    start=not sup_started[t], stop=stop)
                sup_started[t] = True

            # ---- jump 0 on PE: only src-half-0 of u_0 is non-zero ----
            warm(WARM2)
            pq = psum.tile([128, 2, BH], f32, tag="pq", name="pq", bufs=3)
            for mt in range(2):
                ms = slice(mt * 128, (mt + 1) * 128)
                rmm(pq[:, mt, :], el[0][:, ms], upv0[:, 0, :],
                    start=True, stop=False)
                rmm(pq[:, mt, :], er[0][:, ms], upv0[:, 1, :],
                    start=False, stop=True)
            sup_acc(upv0, 0)

            # DVE: upv_1 src-half-0 can go as soon as pq_0 lands
            upv = [state.tile([128, 2, BH], f32r, tag=f"upv{t}", name=f"upv{t}")
                   for t in range(2)]
            nc.vector.tensor_tensor(
                out=upv[0][:], in0=c01[0][:],
                in1=pq[:, 0, :][:, None, :].broadcast_to([128, 2, BH]), op=mult)
            nc.vector.tensor_tensor(
                out=upv[1][:], in0=c01[1][:],
                in1=pq[:, 1, :][:, None, :].broadcast_to([128, 2, BH]), op=mult)

            # ---- jumps 1..6: pq_1 .. pq_6 (u_2 .. u_7) ----
            # per jump: 8 matmuls (2 dst halves x 2 src halves x 2 matrices)
            # + 2 identity matmuls accumulating this jump's upv into psup;
            # front 4 need only upv[src-half-0]
            for j in range(1, J - 1):
                pq_new = psum.tile([128, 2, BH], f32, tag="pq", name="pq",
                                   bufs=3)
                for mt in range(2):
                    ms = slice(mt * 128, (mt + 1) * 128)
                    rmm(pq_new[:, mt, :], el[0][:, ms], upv[0][:, 0],
                        start=True, stop=False)
                    rmm(pq_new[:, mt, :], er[0][:, ms], upv[0][:, 1],
                        start=False, stop=False)
                    rmm(pq_new[:, mt, :], el[1][:, ms], upv[1][:, 0],
                        start=False, stop=False)
                    rmm(pq_new[:, mt, :], er[1][:, ms], upv[1][:, 1],
                        start=False, stop=True)
                # this jump's state joins the running sum (identity weights
                # stay loaded across the consecutive pair)
                sup_acc(upv[0], 0)
                sup_acc(upv[1], 1)

                upv_new = [state.tile([128, 2, BH], f32r, tag=f"upv{t}",
                                      name=f"upv{t}") for t in range(2)]
                nc.vector.tensor_tensor(
                    out=upv_new[0][:], in0=c01[0][:],
                    in1=pq_new[:, 0, :][:, None, :].broadcast_to([128, 2, BH]),
                    op=mult)
                nc.vector.tensor_tensor(
                    out=upv_new[1][:], in0=c01[1][:],
                    in1=pq_new[:, 1, :][:, None, :].broadcast_to([128, 2, BH]),
                    op=mult)
                upv = upv_new
                pq = pq_new

            # final state u_7 closes both accumulators
            sup_acc(upv[0], 0, stop=True)
            sup_acc(upv[1], 1, stop=True)

            # ---- leaf block: leaf = C @ sum_j u_j via sup = sum_j upv_j ----
            supc = [state.tile([128, 2, BH], f32r, tag=f"supc{t}",
                               name=f"supc{t}", bufs=1) for t in range(2)]
            for t in range(2):
                nc.vector.tensor_copy(supc[t][:], psup[t][:])
            pleaf = psum1.tile([BH, 256], f32, tag="pl", name="pl")
            rmm(pleaf[:], supc[0][:, 0], el[0][:, 256:512], start=True, stop=False)
            rmm(pleaf[:], supc[1][:, 0], el[1][:, 256:512], start=False, stop=False)
            rmm(pleaf[:], supc[0][:, 1], er[0][:, 256:512], start=False, stop=False)
            rmm(pleaf[:], supc[1][:, 1], er[1][:, 256:512], start=False, stop=True)

            # ---- output ----
            o = work.tile([BH, 256], f32, tag="o", name="o")
            nc.scalar.copy(o[:], pleaf[:])
            nc.sync.dma_start(p_out[:, :], o[:])

    nc.finalize()
    return nc


def _get_program():
    if "nc" not in _CACHE:
        _CACHE["nc"] = _build_program()
    return _CACHE["nc"]


def _prep_inputs(x, W, bias, M_left, M_right):
    """Host-side shard + layout prep. Core c -> graph c//2, batch half c%2."""
    in_maps = []
    mlt_g, mrt_g, wt_g = [], [], []
    for g in range(G):
        mlt = np.zeros((256, 512), np.float32)
        mrt = np.zeros((256, 512), np.float32)
        tl = np.ascontiguousarray(M_left[g].T)   # (255, 511)
        tr = np.ascontiguousarray(M_right[g].T)
        for dst, src in ((mlt, tl), (mrt, tr)):
            dst[0:255, 0:255] = src[:, 0:255]
            dst[0:255, 256:512] = src[:, 255:511]
            dst[0:255, 255] = NEG
        mlt_g.append(mlt)
        mrt_g.append(mrt)
        wt = np.zeros((512, 256), np.float32)
        wt[:, 0:255] = W[g].T
        wt_g.append(wt)
    xt_h = [np.ascontiguousarray(x[h * BH:(h + 1) * BH].T) for h in range(2)]
    for c in range(NCORES):
        g, h = c // 2, c % 2
        wxb = np.zeros((128, 1280), np.float32)
        wt, xt = wt_g[g], xt_h[h]
        for k in range(4):
            wxb[:, k * 320:k * 320 + 256] = wt[k * 128:(k + 1) * 128, :]
            wxb[:, k * 320 + 256:k * 320 + 320] = xt[k * 128:(k + 1) * 128, :]
        bp2 = np.zeros((128, 2), np.float32)
        bp2[:, 0] = -bias[g][0:128]
        bp2[0:127, 1] = -bias[g][128:255]
        in_maps.append({"mlt": mlt_g[g], "mrt": mrt_g[g], "wxb": wxb,
                        "bp2": bp2, "idn": np.eye(128, dtype=np.float32)})
    return in_maps


def _assemble(results):
    eps = np.float32(1e-5)
    ret = np.empty((B, L, G), np.float32)
    for c in range(NCORES):
        g, h = c // 2, c % 2
        ret[h * BH:(h + 1) * BH, :, g] = results[c]["out"]
    ret = np.where(ret > 0.0, ret, eps)
    ret = np.where(ret < 1.0, ret, np.float32(1.0) - eps)
    return ret.astype(np.float32)


def run_on_device(in_maps, trace=False, **kw):
    from concourse.bass_utils import run_bass_kernel_spmd
    nc = _get_program()
    return run_bass_kernel_spmd(nc, in_maps, list(range(NCORES)), trace=trace, **kw)


def kernel(x, W, bias, M_left, M_right):
    in_maps = _prep_inputs(
        np.asarray(x, np.float32), np.asarray(W, np.float32),
        np.asarray(bias, np.float32), np.asarray(M_left, np.float32),
        np.asarray(M_right, np.float32),
    )
    res = run_on_device(in_maps)
    return _assemble(res.results)


# revision 62
# speedup vs baseline: 1.1468x; 1.1015x over previous
"""Trainium2 Bass kernel for nn_Graphs (soft decision-graph probability propagation).

Reference math (G=4 graphs, B=128 batch, N=255 internal nodes, L=256 leaves,
F=512 features, J=8 jumps):
  b  = sigmoid(x @ W_g^T + bias_g)                  (per graph: B x N)
  M0 = softmax(M_left, axis=dest), M1 = softmax(M_right, axis=dest)
  q  = [b*(M1-M0)+M0 | leaf-identity]               (per (g,batch): 511x511)
  prob <- q @ prob, J times, starting from e0; return leaf probs.

Restructure (v1):
  - q never materialized. With u = prob[internal], one jump is
      u' = E0 @ (c0*u) + E1 @ (c1*u),  c0 = r0*(1-b), c1 = r1*b
    where E0/E1 = exp(M^T) (softmax denominators r folded into c).
  - Leaf rows only accumulate:  leaf = E0L @ (sum_j c0*u_j) + E1L @ (sum_j c1*u_j)
    -> the leaf matmuls are hoisted OUT of the jump loop entirely; the
    running sums (supv) are kept by the otherwise-idle gpsimd engine.
  - Jump 0's state is the one-hot e0, so its 8 matmuls collapse to 4
    rank-1 matmuls reading row 0 of c01 directly.
  - bias is folded into the b-matmul as a rank-1 (bias x ones) accumulate,
    removing the bias DMA + activation bias dependency.

Scheduling (v1):
  - DMA issue (DIRECT2D ~650ns/instr, serialized per sequencer) is split
    across BOTH hwdge rings: Act ring carries the two src-half-0 M tiles
    (needed first by the exp stream), SP ring carries wxb + the two
    src-half-1 M tiles. gpsimd issues no DMAs (avoids the 5us SWDGE drain).
  - A dummy activation triggers the ACT_TABLE_LOAD at t~0 so the exp
    stream starts the moment the first M tile lands.
  - Row sums run on DVE (tensor_reduce) instead of the Act accumulator,
    cutting the serial Act-engine time by ~1.5us.
  - PE warm-up matmuls (bf16, no deps) keep the PE clock ramped through
    the prefix; tuned so the jump chain runs at the 2.4GHz p-state.
  - Generous tile-pool buffering (no tag reuse for one-shot tiles) keeps
    instructions at <=1 semaphore wait, minimizing bacc event semaphores
    (the end-of-program event-sem drain is ~150ns per event sem).

Sharding: 8 cores = (graph g = core//2) x (batch half h = core%2, 64 rows).
Output per core: (64,256) batch-major leaf probs; host assembles (B,L,G)
and applies the reference interval clamp.
"""

import numpy as np

G, B, N, L, F, J = 4, 128, 255, 256, 512, 8
BH = B // 2  # 64 batch rows per core
NCORES = 8
NEG = np.float32(-1e4)

_CACHE = {}

# PE warm-up budget (tuned from traces): bf16 512-row matmuls ~216ns at peak.
# The tile scheduler interleaves these dependency-free matmuls into PE idle
# gaps, keeping the clock ramped until the jump chain starts.
WARM1 = 20  # before the b-phase matmuls
WARM2 = 18  # between b-phase and jump 0


def _build_program():
    import concourse.mybir as mybir
    from concourse import bacc
    from concourse.tile import TileContext

    f32 = mybir.dt.float32
    bf16 = mybir.dt.bfloat16
    f32r = mybir.dt.float32r  # single-pass fp32 matmul mode
    AF = mybir.ActivationFunctionType
    AX = mybir.AxisListType
    mult = mybir.AluOpType.mult
    addop = mybir.AluOpType.add

    nc = bacc.Bacc(None)
    p_m0 = nc.declare_dram_parameter("m0", [128, 1024], f32, isOutput=False)
    p_m1 = nc.declare_dram_parameter("m1", [128, 1024], f32, isOutput=False)
    p_wxb = nc.declare_dram_parameter("wxb", [128, 1280], f32r, isOutput=False)
    p_bp2 = nc.declare_dram_parameter("bp2", [128, 2], f32, isOutput=False)
    p_idn = nc.declare_dram_parameter("idn", [128, 128], f32r, isOutput=False)
    p_out = nc.declare_dram_parameter("out", [BH, 256], f32, isOutput=True)

    # dtype discipline (measured on hw): f32r data is exact ONLY through the
    # PE; any DVE/gpsimd/Act READ of f32r degrades to ~bf16. So f32r tiles
    # (wxb, el/er, upv, sup) are consumed exclusively by matmuls; every
    # value another engine needs (row sums via accum_out, pq in PSUM) stays
    # plain f32.
    def rmm(out, lhsT, rhs, **kw):
        nc.tensor.matmul(out, lhsT, rhs, **kw)

    with TileContext(nc) as tc:
        with (
            tc.tile_pool(name="consts", bufs=1) as consts,
            tc.tile_pool(name="work", bufs=1) as work,
            tc.tile_pool(name="state", bufs=7) as state,
            tc.tile_pool(name="psum", bufs=1, space="PSUM") as psum,
            tc.tile_pool(name="psum1", bufs=1, space="PSUM") as psum1,
        ):
            # ---- early consts (gpsimd memsets; keeps DVE/Act/PE free) ----
            dumw = consts.tile([128, 1], f32, tag="dumw", name="dumw")
            nc.gpsimd.memset(dumw[:], 0.0)
            wsc = consts.tile([128, 128], bf16, tag="wsc", name="wsc")
            rsc = consts.tile([128, 512], bf16, tag="rsc", name="rsc")
            nc.gpsimd.memset(wsc[:], 0.0)
            nc.gpsimd.memset(rsc[:], 0.0)
            u32 = mybir.dt.uint32
            # jump-0 state (only src row 0 non-zero; src half 1 is all zero
            # and its matmuls are skipped). f32r: read only by the PE.
            upv0 = state.tile([128, 2, BH], f32r, tag="u0seed", name="upv0",
                              bufs=1)
            nc.gpsimd.memset(upv0[:].bitcast(u32), 0)

            # trigger the ACT_TABLE_LOAD for Exp immediately (runs on the Act
            # ENGINE while the Act SEQUENCER generates the DMA descriptors
            # below)
            dumo = work.tile([128, 1], f32, tag="dumo", name="dumo")
            nc.scalar.activation(dumo[:], dumw[:], AF.Exp)

            # ---- input DMAs ----
            # Big transfers all on the SP hwdge ring (splitting across both
            # rings starves each; measured). Each dma_start costs ~650ns of
            # descriptor-gen on its sequencer plus ~1.5us completion latency
            # regardless of size, so the M matrices go as TWO packed
            # (128,1024) transfers: mraw[t] = [mlt src-half-t | mrt src-half-t].
            # Tiny transfers ride the Act ring, overlapping the table load.
            mraw = [consts.tile([128, 1024], f32, tag=f"mraw{t}",
                                name=f"mraw{t}") for t in range(2)]
            wxb = consts.tile([128, 1280], f32r, tag="wxb", name="wxb")
            nc.sync.dma_start(mraw[0][:], p_m0[:, :])
            nc.sync.dma_start(wxb[:], p_wxb[:, :])
            nc.sync.dma_start(mraw[1][:], p_m1[:, :])
            bp2 = consts.tile([128, 2], f32, tag="bp2", name="bp2")
            nc.scalar.dma_start(bp2[:], p_bp2[:, :])
            idn = consts.tile([128, 128], f32r, tag="idn", name="idn")
            nc.scalar.dma_start(idn[:], p_idn[:, :])

            # ---- PE warm-up ----
            pwarm = psum1.tile([128, 512], f32, tag="pwarm", name="pwarm")

            def warm(n):
                for _ in range(n):
                    nc.tensor.matmul(pwarm[:], wsc[:], rsc[:], start=True, stop=True)

            # ---- exp(M^T), lazily normalized ----
            el = [consts.tile([128, 512], f32r, tag=f"el{t}", name=f"el{t}")
                  for t in range(2)]
            er = [consts.tile([128, 512], f32r, tag=f"er{t}", name=f"er{t}")
                  for t in range(2)]

            # exp with inline row-sum accumulation (Act's accumulator is
            # fp32-exact; the f32r-tagged exp output can't be summed by DVE)
            psums = {}

            def exps(dst, src, i):
                ps = [work.tile([128, 1], f32, tag=f"ps{i}_{hh}",
                                name=f"ps{i}_{hh}") for hh in range(2)]
                for hh in range(2):
                    sl = slice(hh * 256, (hh + 1) * 256)
                    nc.scalar.activation(dst[:, sl], src[:, sl], AF.Exp,
                                         accum_out=ps[hh][:])
                psums[i] = ps

            # Act order: src-half-0 exps, then eb (fills the wait for mraw1),
            # then src-half-1 exps.
            exps(el[0], mraw[0][:, 0:512], 0)
            exps(er[0], mraw[0][:, 512:1024], 2)

            # ---- b-phase on PE (also part of clock ramp) ----
            warm(WARM1)
            # pbb shares the pq0 psum slot rotation (it is long dead before
            # the slot comes around again)
            pbb = psum.tile([128, 2, BH], f32, tag="pq0", name="pbb", bufs=2)
            for mh in range(2):
                for k in range(4):
                    rmm(pbb[:, mh, :],
                        wxb[:, k * 320 + mh * 128:k * 320 + (mh + 1) * 128],
                        wxb[:, k * 320 + 256:k * 320 + 320],
                        start=(k == 0), stop=(k == 3))

            # eb = exp(-(logit)) = exp(pb*-1 + (-bias)); bp2 carries -bias
            eb = [work.tile([128, BH], f32, tag=f"eb{mh}", name=f"eb{mh}")
                  for mh in range(2)]
            for mh in range(2):
                nc.scalar.activation(eb[mh][:], pbb[:, mh, :], AF.Exp,
                                     bias=bp2[:, mh:mh + 1], scale=-1.0)

            exps(el[1], mraw[1][:, 0:512], 1)
            exps(er[1], mraw[1][:, 512:1024], 3)

            # ---- reciprocals of row sums on DVE; c01 packing ----
            # c01[mh][:,0] = r0*(1-b), c01[mh][:,1] = r1*b
            def recip_of(i):
                ps = psums[i]
                red = work.tile([128, 1], f32, tag=f"red{i}", name=f"red{i}")
                nc.vector.tensor_add(red[:], ps[0][:], ps[1][:])
                r = consts.tile([128, 1], f32, tag=f"rec{i}", name=f"rec{i}")
                nc.vector.reciprocal(r[:], red[:])
                return r

            c01 = [consts.tile([128, 2, BH], f32, tag=f"c01{t}", name=f"c01{t}")
                   for t in range(2)]

            def c01_chain(mh, r0, r1):
                den = work.tile([128, BH], f32, tag=f"den{mh}", name=f"den{mh}")
                nc.vector.tensor_scalar_add(den[:], eb[mh][:], 1.0)
                sig = work.tile([128, BH], f32, tag=f"sig{mh}", name=f"sig{mh}")
                nc.vector.reciprocal(sig[:], den[:])
                nc.vector.tensor_scalar_mul(c01[mh][:, 1], sig[:], r1[:])
                nc.vector.tensor_mul(sig[:], sig[:], eb[mh][:])
                nc.vector.tensor_scalar_mul(c01[mh][:, 0], sig[:], r0[:])

            rec0 = recip_of(0)
            rec2 = recip_of(2)
            c01_chain(0, rec0, rec2)

            # jump-0 state: row 0 of c01[0] (DVE rounds it to f32r for the PE)
            nc.vector.tensor_copy(upv0[0:1, :, :], c01[0][0:1, :, :])

            rec1 = recip_of(1)
            rec3 = recip_of(3)
            c01_chain(1, rec1, rec3)

            # sum_j upv_j is accumulated EXACTLY in PSUM by identity matmuls
            # (0/1 weights are exact in the PE's f32r path; u_j decays with j
            # so, unlike a prefix-sum recursion, fp noise does not amplify)
            psup = [psum.tile([128, 2, BH], f32, tag=f"psup{t}",
                              name=f"psup{t}") for t in range(2)]
            sup_started = [False, False]

            def sup_acc(upv_t, t, stop=False):
                rmm(psup[t][:], idn[:], upv_t[:],
                    start=not sup_started[t], stop=stop)
                sup_started[t] = True

            # pq as two single-bank tiles (one accumulation group per bank --
            # same-bank groups must not interleave, cross-bank is fine)
            def pq_pair():
                return [psum.tile([128, BH], f32, tag=f"pq{mt}",
                                  name=f"pq{mt}", bufs=2) for mt in range(2)]

            # ---- jump 0 on PE: only src-half-0 of u_0 is non-zero ----
            warm(WARM2)
            pq = pq_pair()
            for mt in range(2):
                ms = slice(mt * 128, (mt + 1) * 128)
                rmm(pq[mt][:], el[0][:, ms], upv0[:, 0, :],
                    start=True, stop=False)
                rmm(pq[mt][:], er[0][:, ms], upv0[:, 1, :],
                    start=False, stop=True)
            sup_acc(upv0, 0)

            # DVE: upv_1 src-half-0 can go as soon as pq_0 lands
            upv = [state.tile([128, 2, BH], f32r, tag=f"upv{t}", name=f"upv{t}")
                   for t in range(2)]
            nc.vector.tensor_tensor(
                out=upv[0][:], in0=c01[0][:],
                in1=pq[0][:, None, :].broadcast_to([128, 2, BH]), op=mult)
            nc.vector.tensor_tensor(
                out=upv[1][:], in0=c01[1][:],
                in1=pq[1][:, None, :].broadcast_to([128, 2, BH]), op=mult)

            # ---- jumps 1..6: pq_1 .. pq_6 (u_2 .. u_7) ----
            # per jump: 8 matmuls + 2 identity matmuls; the 4 matmuls that
            # need only upv[src-half-0] (and upv[0]'s identity matmul) go
            # first so the PE restarts before upv[src-half-1] is ready
            for j in range(1, J - 1):
                pq_new = pq_pair()
                for mt in range(2):
                    ms = slice(mt * 128, (mt + 1) * 128)
                    rmm(pq_new[mt][:], el[0][:, ms], upv[0][:, 0],
                        start=True, stop=False)
                    rmm(pq_new[mt][:], er[0][:, ms], upv[0][:, 1],
                        start=False, stop=False)
                sup_acc(upv[0], 0)
                for mt in range(2):
                    ms = slice(mt * 128, (mt + 1) * 128)
                    rmm(pq_new[mt][:], el[1][:, ms], upv[1][:, 0],
                        start=False, stop=False)
                    rmm(pq_new[mt][:], er[1][:, ms], upv[1][:, 1],
                        start=False, stop=True)
                sup_acc(upv[1], 1)

                upv_new = [state.tile([128, 2, BH], f32r, tag=f"upv{t}",
                                      name=f"upv{t}") for t in range(2)]
                nc.vector.tensor_tensor(
                    out=upv_new[0][:], in0=c01[0][:],
                    in1=pq_new[0][:, None, :].broadcast_to([128, 2, BH]),
                    op=mult)
                nc.vector.tensor_tensor(
                    out=upv_new[1][:], in0=c01[1][:],
                    in1=pq_new[1][:, None, :].broadcast_to([128, 2, BH]),
                    op=mult)
                upv = upv_new
                pq = pq_new

            # final state u_7 closes both accumulators
            sup_acc(upv[0], 0, stop=True)
            sup_acc(upv[1], 1, stop=True)

            # ---- leaf block: leaf = C @ sum_j u_j via sup = sum_j upv_j ----
            supc = [state.tile([128, 2, BH], f32r, tag=f"supc{t}",
                               name=f"supc{t}", bufs=1) for t in range(2)]
            for t in range(2):
                nc.vector.tensor_copy(supc[t][:], psup[t][:])
            pleaf = psum1.tile([BH, 256], f32, tag="pl", name="pl")
            rmm(pleaf[:], supc[0][:, 0], el[0][:, 256:512], start=True, stop=False)
            rmm(pleaf[:], supc[1][:, 0], el[1][:, 256:512], start=False, stop=False)
            rmm(pleaf[:], supc[0][:, 1], er[0][:, 256:512], start=False, stop=False)
            rmm(pleaf[:], supc[1][:, 1], er[1][:, 256:512], start=False, stop=True)

            # ---- output ----
            o = work.tile([BH, 256], f32, tag="o", name="o")
            nc.scalar.copy(o[:], pleaf[:])
            nc.sync.dma_start(p_out[:, :], o[:])

    nc.finalize()
    return nc


def _get_program():
    if "nc" not in _CACHE:
        _CACHE["nc"] = _build_program()
    return _CACHE["nc"]


def _prep_inputs(x, W, bias, M_left, M_right):
    """Host-side shard + layout prep. Core c -> graph c//2, batch half c%2."""
    in_maps = []
    mlt_g, mrt_g, wt_g = [], [], []
    for g in range(G):
        mlt = np.zeros((256, 512), np.float32)
        mrt = np.zeros((256, 512), np.float32)
        tl = np.ascontiguousarray(M_left[g].T)   # (255, 511)
        tr = np.ascontiguousarray(M_right[g].T)
        for dst, src in ((mlt, tl), (mrt, tr)):
            dst[0:255, 0:255] = src[:, 0:255]
            dst[0:255, 256:512] = src[:, 255:511]
            dst[0:255, 255] = NEG
        mlt_g.append(mlt)
        mrt_g.append(mrt)
        wt = np.zeros((512, 256), np.float32)
        wt[:, 0:255] = W[g].T
        wt_g.append(wt)
    xt_h = [np.ascontiguousarray(x[h * BH:(h + 1) * BH].T) for h in range(2)]
    for c in range(NCORES):
        g, h = c // 2, c % 2
        wxb = np.zeros((128, 1280), np.float32)
        wt, xt = wt_g[g], xt_h[h]
        for k in range(4):
            wxb[:, k * 320:k * 320 + 256] = wt[k * 128:(k + 1) * 128, :]
            wxb[:, k * 320 + 256:k * 320 + 320] = xt[k * 128:(k + 1) * 128, :]
        bp2 = np.zeros((128, 2), np.float32)
        bp2[:, 0] = -bias[g][0:128]
        bp2[0:127, 1] = -bias[g][128:255]
        m0 = np.concatenate([mlt_g[g][0:128], mrt_g[g][0:128]], axis=1)
        m1 = np.concatenate([mlt_g[g][128:256], mrt_g[g][128:256]], axis=1)
        in_maps.append({"m0": np.ascontiguousarray(m0),
                        "m1": np.ascontiguousarray(m1), "wxb": wxb,
                        "bp2": bp2, "idn": np.eye(128, dtype=np.float32)})
    return in_maps


def _assemble(results):
    eps = np.float32(1e-5)
    ret = np.empty((B, L, G), np.float32)
    for c in range(NCORES):
        g, h = c // 2, c % 2
        ret[h * BH:(h + 1) * BH, :, g] = results[c]["out"]
    ret = np.where(ret > 0.0, ret, eps)
    ret = np.where(ret < 1.0, ret, np.float32(1.0) - eps)
    return ret.astype(np.float32)


def run_on_device(in_maps, trace=False, **kw):
    from concourse.bass_utils import run_bass_kernel_spmd
    nc = _get_program()
    return run_bass_kernel_spmd(nc, in_maps, list(range(NCORES)), trace=trace, **kw)


def kernel(x, W, bias, M_left, M_right):
    in_maps = _prep_inputs(
        np.asarray(x, np.float32), np.asarray(W, np.float32),
        np.asarray(bias, np.float32), np.asarray(M_left, np.float32),
        np.asarray(M_right, np.float32),
    )
    res = run_on_device(in_maps)
    return _assemble(res.results)
